# revision 1
# baseline (speedup 1.0000x reference)
import math

import numpy as np

# nn_DescLayer: LayerNorm -> x@M^T, x@R^T -> Nk[b,s,i] = sum_{j,g} P[i,j,g] *
# cos(2*pi*k[b,s]/periods[i,j,g]) * xproj[b,s,j]; out = res + Nk.
# Data-parallel over the 1024 (b,s) tokens: 128 tokens per NeuronCore.
#
# Per-core layout: periods flattened as flat = i*512 + j*8 + g are viewed as
# (p, i, glo) with p = j*2 + (g>>2) on partitions and (i, glo=g&3) in the
# free dim (flat = i*512 + p*4 + glo). Then xproj[t, j] is constant per
# partition (j = p>>1), so one fused scalar_tensor_tensor computes
# (sin ⊙ xp) ⊙ P per token, and the (j,g)-contraction is a plain
# partition-sum on the TensorEngine: stationary = ones(128,1) (loaded once),
# moving = fp16 product tiles batched 8 tokens x 64 i x 4 glo-accumulated
# matmuls into PSUM rows that are already in flat y order.
#
# cos(2pi*k/p) = sin(2pi*k/p + pi/2) is valid for the Sin LUT ([-pi, pi])
# whenever k/p <= 1/4, i.e. for all periods with i >= 4. The first 2048
# periods (i < 4) go through a token-major pass with explicit
# round-to-nearest range reduction: f = (k/p + 1/4) - round(k/p + 1/4),
# cos = sin(2pi*f).

B, S, D, NB = 2, 512, 64, 8
N_CORES = 8
TOK = (B * S) // N_CORES  # 128 tokens per core
NPER = D * D * NB  # 32768 periods
SMALL = 2048  # flat period idx < SMALL (i<4) needs range reduction
LN_EPS = 1e-5
TWO_PI = 2.0 * math.pi
RND_C = 12582912.0  # 1.5 * 2**23: (u + C) - C == round-to-nearest(u) in f32

GRP = 8  # tokens per PE row-matmul group
PATH_B = 0  # tokens per group whose sin angle is computed on VectorE
USE_STT = False  # fused (sin*xp)*P in one scalar_tensor_tensor
MAIN_16 = True  # fp16 for sin/P/prod tiles

_CACHE = {}


def _split_waits(nc, maxw=1):
    """This walrus build rejects instructions carrying more than one sem
    wait. Hoist excess waits onto same-engine NoOps placed immediately
    before the instruction (same engine stream => executes first)."""
    import bass_rust
    import concourse.mybir as mybir

    ctr = [0]
    for f in nc.m.functions:
        for b in f.blocks:
            new_insts = []
            changed = False
            for inst in b.instructions:
                si = inst.sync_info
                waits = list(si.on_wait) if si and si.on_wait else []
                if len(waits) > maxw:
                    keep = waits[-maxw:]
                    extra = waits[:-maxw]
                    for i0 in range(0, len(extra), maxw):
                        ctr[0] += 1
                        nop = bass_rust.InstNoOp(
                            name=f"I-waitsplit-{ctr[0]}",
                            engine=inst.engine,
                            text_hint="waitsplit",
                            sync_info=mybir.SyncInfo(
                                on_wait=extra[i0 : i0 + maxw], on_update=[]
                            ),
                        )
                        new_insts.append(nop)
                    si.on_wait = keep
                    changed = True
                new_insts.append(inst)
            if changed:
                b.instructions = new_insts


def _build_program(split=True):
    import concourse.bass as bass
    import concourse.mybir as mybir
    from concourse.tile import TileContext
    from concourse.vector_clock import ScopedClock, VectorClock

    # --- workaround: walrus rejects >1 sem wait on the Tile tail drain;
    # spread the waits over SP nops (1 each), then issue a bare drain.
    def _drain_and_barrier(self, tick_clock, wait_clock):
        nc = self.nc
        gc = tick_clock.global_clock
        n = len(gc)
        for i in range(n):
            tick = gc[i]
            if tick <= 0:
                continue
            vec = [0] * n
            vec[i] = tick
            nop_inst = nc.sync.nop(nofuse=True, hint=f"drain_wait_{i}")
            wait_clock.add_sem_waits(
                nop_inst.ins, ScopedClock({None: VectorClock(vec)})
            )
        nc.sync.drain()
        nc.all_engine_barrier()
        assert self.sems is not None
        popped = nc._tile_sem_poison_stack.pop()
        assert popped is self._sem_poison
        nc.clear_and_free_semaphores(list(self.sems.allocated().values()))
        nc.all_engine_barrier()

    TileContext._drain_and_barrier = _drain_and_barrier

    f32 = mybir.dt.float32
    f16 = mybir.dt.float16 if MAIN_16 else mybir.dt.float32
    i32 = mybir.dt.int32
    AF = mybir.ActivationFunctionType
    OP = mybir.AluOpType
    AX = mybir.AxisListType

    nc = bass.Bass()
    X = nc.declare_dram_parameter("x", [TOK, D], f32, isOutput=False)
    K = nc.declare_dram_parameter("k", [1, TOK], f32, isOutput=False)
    Mw = nc.declare_dram_parameter("M", [D, D], f32, isOutput=False)
    Rw = nc.declare_dram_parameter("R", [D, D], f32, isOutput=False)
    Pw = nc.declare_dram_parameter("P", [NPER], f32, isOutput=False)
    Gam = nc.declare_dram_parameter("gamma", [1, D], f32, isOutput=False)
    Bet = nc.declare_dram_parameter("beta", [1, D], f32, isOutput=False)
    Per = nc.declare_dram_parameter("periods", [NPER], f32, isOutput=False)
    Ones = nc.declare_dram_parameter("ones", [1, 128], f32, isOutput=False)
    Rep = nc.declare_dram_parameter("rep", [D, 128], f32, isOutput=False)
    Idm = nc.declare_dram_parameter("idm", [128, 128], f32, isOutput=False)
    Y = nc.declare_dram_parameter("y", [TOK, D], f32, isOutput=True)

    NGRP = TOK // GRP

    with TileContext(nc) as tc:
        with (
            tc.tile_pool(name="const", bufs=1) as cp,
            tc.tile_pool(name="ang", bufs=3) as angp,
            tc.tile_pool(name="sino", bufs=3) as sinp,
            tc.tile_pool(name="prod", bufs=3) as prodp,
            tc.tile_pool(name="prow", bufs=4, space="PSUM") as prowp,
            tc.tile_pool(name="pprep", bufs=3, space="PSUM") as pprep,
            tc.tile_pool(name="dram", bufs=1, space="DRAM") as dramp,
        ):
            scratch = dramp.tile([TOK * D], f32, tag="scr")
            # ---------------- load constants ----------------
            xs = cp.tile([TOK, D], f32, tag="xs")
            nc.sync.dma_start(out=xs[:], in_=X[:])
            kr = cp.tile([1, TOK], f32, tag="kr")
            nc.sync.dma_start(out=kr[:], in_=K[:])
            mn = cp.tile([D, D], f32, tag="mn")
            nc.sync.dma_start(out=mn[:], in_=Mw[:])
            rn = cp.tile([D, D], f32, tag="rn")
            nc.sync.dma_start(out=rn[:], in_=Rw[:])
            gam = cp.tile([1, D], f32, tag="gam")
            nc.sync.dma_start(out=gam[:], in_=Gam[:])
            bet = cp.tile([1, D], f32, tag="bet")
            nc.sync.dma_start(out=bet[:], in_=Bet[:])
            onesr = cp.tile([1, 128], f32, tag="onesr")
            nc.sync.dma_start(out=onesr[:], in_=Ones[:])
            rep = cp.tile([D, 128], f32, tag="rep")
            nc.sync.dma_start(out=rep[:], in_=Rep[:])
            idm = cp.tile([128, 128], f32, tag="idm")
            nc.sync.dma_start(out=idm[:], in_=Idm[:])

            # periods/P in (p, i, glo) layout: flat = i*512 + p*4 + glo
            pert2 = cp.tile([128, 256], f32, tag="pert2")
            nc.sync.dma_start(
                out=pert2[:].rearrange("p (i glo) -> p i glo", glo=4),
                in_=Per[:].rearrange("(i p glo) -> p i glo", i=D, p=128, glo=4),
            )
            pt2 = cp.tile([128, 256], f32, tag="pt2")
            nc.sync.dma_start(
                out=pt2[:].rearrange("p (i glo) -> p i glo", glo=4),
                in_=Pw[:].rearrange("(i p glo) -> p i glo", i=D, p=128, glo=4),
            )
            invp2 = cp.tile([128, 256], f32, tag="invp2")
            nc.vector.reciprocal(invp2[:], pert2[:])
            # copy with i<4 zeroed for the main-path activations: keeps every
            # sin argument inside the LUT range [-pi, pi] (those columns'
            # products are zeroed via p2h anyway; sin(pi/2)=1 is harmless)
            invp2c = cp.tile([128, 256], f32, tag="invp2c")
            nc.vector.tensor_copy(invp2c[:], invp2[:])
            nc.vector.tensor_scalar(
                invp2c[:, 0:16], invp2c[:, 0:16], 0.0, None, OP.mult
            )
            invp2g = cp.tile([128, 256], f32, tag="invp2g")
            nc.vector.tensor_copy(
                invp2g[:].rearrange("p (glo i) -> p i glo", glo=4),
                invp2c[:].rearrange("p (i glo) -> p i glo", glo=4),
            )

            # P as fp16 in (glo, i) order with i<4 zeroed (small-p pass owns
            # those; zeroing also kills the dummy sin values there). The
            # (glo, i) order makes each glo slice of the product tile a
            # contiguous, 4B-aligned matmul moving operand.
            p2hg = cp.tile([128, 256], f16, tag="p2hg")
            nc.vector.tensor_copy(
                p2hg[:].rearrange("p (glo i) -> p i glo", glo=4),
                pt2[:].rearrange("p (i glo) -> p i glo", glo=4),
            )
            p2hg3 = p2hg[:].rearrange("p (glo i) -> p glo i", glo=4)
            nc.vector.tensor_scalar(
                p2hg3[:, :, 0:4], p2hg3[:, :, 0:4], 0.0, None, OP.mult
            )

            p0r = cp.tile([1, SMALL], f32, tag="p0r")
            nc.sync.dma_start(
                out=p0r[:], in_=Pw[0:SMALL].rearrange("(a b) -> a b", a=1)
            )

            # activation per-partition bias vectors
            bias_hp = cp.tile([128, 1], f32, tag="bias_hp")
            nc.vector.memset(bias_hp[:], math.pi / 2.0)
            bias_z = cp.tile([128, 1], f32, tag="bias_z")
            nc.vector.memset(bias_z[:], 0.0)
            ones_h = cp.tile([128, 1], f16, tag="ones_h")
            nc.vector.memset(ones_h[:], 1.0)

            # ---------------- PE prep ----------------
            id64 = idm[0:64, 0:64]

            mt_ps = pprep.tile([D, D], f32, tag="pp")
            nc.tensor.transpose(mt_ps[:], mn[:], id64)
            mt = cp.tile([D, D], f32, tag="mt")
            nc.vector.tensor_copy(mt[:], mt_ps[:])

            rt_ps = pprep.tile([D, D], f32, tag="pp")
            nc.tensor.transpose(rt_ps[:], rn[:], id64)
            rt = cp.tile([D, D], f32, tag="rt")
            nc.vector.tensor_copy(rt[:], rt_ps[:])

            gb_ps = pprep.tile([128, D], f32, tag="pp")
            nc.tensor.matmul(gb_ps[:], onesr[:], gam[:], start=True, stop=True)
            gb = cp.tile([128, D], f32, tag="gb")
            nc.vector.tensor_copy(gb[:], gb_ps[:])
            bb_ps = pprep.tile([128, D], f32, tag="pp")
            nc.tensor.matmul(bb_ps[:], onesr[:], bet[:], start=True, stop=True)
            bb = cp.tile([128, D], f32, tag="bb")
            nc.vector.tensor_copy(bb[:], bb_ps[:])

            kv_ps = pprep.tile([TOK, 1], f32, tag="pp")
            nc.tensor.matmul(kv_ps[:], kr[:], onesr[:, 0:1], start=True, stop=True)
            kvec = cp.tile([TOK, 1], f32, tag="kvec")
            nc.vector.tensor_copy(kvec[:], kv_ps[:])

            k2_ps = pprep.tile([128, TOK], f32, tag="pp")
            nc.tensor.matmul(k2_ps[:], onesr[:], kr[:], start=True, stop=True)
            k2pi = cp.tile([128, TOK], f32, tag="k2pi")
            nc.vector.tensor_scalar(k2pi[:], k2_ps[:], TWO_PI, None, OP.mult)

            # ---------------- LayerNorm (token-major) ----------------
            rsum = cp.tile([TOK, 1], f32, tag="rsum")
            nc.vector.tensor_reduce(rsum[:], xs[:], AX.X, OP.add)
            mu = cp.tile([TOK, 1], f32, tag="mu")
            nc.vector.tensor_scalar(mu[:], rsum[:], 1.0 / D, None, OP.mult)
            cen = cp.tile([TOK, D], f32, tag="cen")
            nc.vector.tensor_scalar(cen[:], xs[:], mu[:], None, OP.subtract)
            sq = cp.tile([TOK, D], f32, tag="sq")
            nc.vector.tensor_tensor(sq[:], cen[:], cen[:], OP.mult)
            ssq = cp.tile([TOK, 1], f32, tag="ssq")
            nc.vector.tensor_reduce(ssq[:], sq[:], AX.X, OP.add)
            veps = cp.tile([TOK, 1], f32, tag="veps")
            nc.vector.tensor_scalar(veps[:], ssq[:], 1.0 / D, LN_EPS, OP.mult, OP.add)

            # rstd = 1/sqrt(veps): bit-hack seed + 3 Newton steps (keeps
            # ScalarE's activation table on the trig set only)
            ti = cp.tile([TOK, 1], i32, tag="ti")
            nc.vector.tensor_scalar(
                ti[:], veps[:].bitcast(i32), 1, -1, OP.arith_shift_right,
                OP.bitwise_xor,
            )
            yr = cp.tile([TOK, 1], f32, tag="yr")
            nc.vector.tensor_scalar(
                yr[:].bitcast(i32), ti[:], 0x5F3759DF + 1, None, OP.add
            )
            hh = cp.tile([TOK, 1], f32, tag="hh")
            nc.vector.tensor_scalar(hh[:], veps[:], 0.5, None, OP.mult)
            for it in range(3):
                t1 = cp.tile([TOK, 1], f32, tag=f"nt1_{it}")
                nc.vector.tensor_tensor(t1[:], yr[:], yr[:], OP.mult)
                t2 = cp.tile([TOK, 1], f32, tag=f"nt2_{it}")
                nc.vector.tensor_tensor(t2[:], t1[:], hh[:], OP.mult)
                t3 = cp.tile([TOK, 1], f32, tag=f"nt3_{it}")
                nc.vector.tensor_scalar(t3[:], t2[:], 1.5, -1.0, OP.subtract, OP.mult)
                yn = cp.tile([TOK, 1], f32, tag=f"nt4_{it}")
                nc.vector.tensor_tensor(yn[:], yr[:], t3[:], OP.mult)
                yr = yn

            ln0 = cp.tile([TOK, D], f32, tag="ln0")
            nc.vector.tensor_scalar(ln0[:], cen[:], yr[:], None, OP.mult)
            ln1 = cp.tile([TOK, D], f32, tag="ln1")
            nc.vector.tensor_tensor(ln1[:], ln0[:], gb[:], OP.mult)
            lnf = cp.tile([TOK, D], f32, tag="lnf")
            nc.vector.tensor_tensor(lnf[:], ln1[:], bb[:], OP.add)

            # ---------------- projections ----------------
            lnT_ps = pprep.tile([D, TOK], f32, tag="pp")
            nc.tensor.transpose(lnT_ps[:], lnf[:], idm[:])
            lnT = cp.tile([D, TOK], f32, tag="lnT")
            nc.vector.tensor_copy(lnT[:], lnT_ps[:])

            xpT_ps = pprep.tile([D, TOK], f32, tag="pp")
            nc.tensor.matmul(xpT_ps[:], mt[:], lnT[:], start=True, stop=True)
            xpT = cp.tile([D, TOK], f32, tag="xpT")
            nc.vector.tensor_copy(xpT[:], xpT_ps[:])

            res_ps = pprep.tile([D, TOK], f32, tag="pp")
            nc.tensor.matmul(res_ps[:], rt[:], lnT[:], start=True, stop=True)
            res_sb = cp.tile([D, TOK], f32, tag="res_sb")
            nc.vector.tensor_copy(res_sb[:], res_ps[:])

            # res back to token-major
            resT_ps = pprep.tile([TOK, D], f32, tag="pp")
            nc.tensor.transpose(resT_ps[:], res_sb[:], id64)
            resT = cp.tile([TOK, D], f32, tag="resT")
            nc.vector.tensor_copy(resT[:], resT_ps[:])

            # xproj token-major (for the small-p pass)
            xp_ps = pprep.tile([TOK, D], f32, tag="pp")
            nc.tensor.transpose(xp_ps[:], xpT[:], id64)
            xp_sb = cp.tile([TOK, D], f32, tag="xp_sb")
            nc.vector.tensor_copy(xp_sb[:], xp_ps[:])

            # XPrep2[p, t] = xproj[t, p>>1]
            xpr_ps = pprep.tile([128, TOK], f32, tag="pp")
            nc.tensor.matmul(xpr_ps[:], rep[:], xpT[:], start=True, stop=True)
            xprep = cp.tile([128, TOK], f32, tag="xprep")
            nc.vector.tensor_copy(xprep[:], xpr_ps[:])

            # ---------------- small-p pass (token-major, i<4) ----------------
            # 1/p row gathered from invp2 (already reciprocals), then
            # broadcast to all partitions on GpSimd
            invrow = cp.tile([1, SMALL], f32, tag="invrow")
            for i in range(4):
                nc.sync.dma_start(
                    out=invrow[0:1, i * 512 : (i + 1) * 512].rearrange(
                        "a (p glo) -> a p glo", glo=4
                    ),
                    in_=invp2[:, i * 4 : i * 4 + 4],
                )
            invp0 = cp.tile([128, SMALL], f32, tag="invp0")
            p0rep = cp.tile([128, SMALL], f16, tag="p0rep")
            for ch in range(4):
                sl = slice(ch * 512, (ch + 1) * 512)
                bi_ps = pprep.tile([128, 512], f32, tag="pp")
                nc.tensor.matmul(bi_ps[:], onesr[:], invrow[:, sl], start=True, stop=True)
                nc.vector.tensor_copy(invp0[:, sl], bi_ps[:])
                bp_ps = pprep.tile([128, 512], f32, tag="pp")
                nc.tensor.matmul(bp_ps[:], onesr[:], p0r[:, sl], start=True, stop=True)
                nc.vector.tensor_copy(p0rep[:, sl], bp_ps[:])

            uu = cp.tile([128, SMALL], f32, tag="uu")
            nc.vector.tensor_scalar(uu[:], invp0[:], kvec[:], 0.25, OP.mult, OP.add)
            rr = cp.tile([128, SMALL], f32, tag="rr")
            nc.vector.tensor_scalar(rr[:], uu[:], RND_C, RND_C, OP.add, OP.subtract)
            ff = cp.tile([128, SMALL], f32, tag="ff")
            nc.vector.tensor_tensor(ff[:], uu[:], rr[:], OP.subtract)
            s0 = cp.tile([128, SMALL], f16, tag="s0")
            nc.scalar.activation(s0[:], ff[:], AF.Sin, bias=bias_z[:], scale=TWO_PI)
            prod0 = cp.tile([128, SMALL], f16, tag="prod0")
            nc.vector.tensor_tensor(prod0[:], s0[:], p0rep[:], OP.mult)
            rg = cp.tile([128, 256], f32, tag="rg")
            nc.vector.tensor_reduce(
                rg[:], prod0[:].rearrange("p (a b) -> p a b", b=NB), AX.X, OP.add
            )
            xp4 = cp.tile([128, 256], f32, tag="xp4")
            for cc in range(4):
                nc.vector.tensor_copy(xp4[:, cc * D : (cc + 1) * D], xp_sb[:])
            rgx = cp.tile([128, 256], f32, tag="rgx")
            nc.vector.tensor_tensor(rgx[:], rg[:], xp4[:], OP.mult)
            nksm = cp.tile([128, 4], f32, tag="nksm")
            nc.vector.tensor_reduce(
                nksm[:], rgx[:].rearrange("p (a b) -> p a b", b=D), AX.X, OP.add
            )

            # ---------------- main loop: groups of GRP tokens ----------------
            out_sb = cp.tile([TOK, D], f32, tag="out_sb")
            rows_sb = cp.tile([1, TOK * D], f32, tag="rows_sb")
            nb = PATH_B
            na = GRP - nb
            for g in range(NGRP):
                t0 = g * GRP
                sino = sinp.tile([128, GRP * 256], f16, tag="sino")
                if nb > 0:
                    ang = angp.tile([128, nb * 256], f32, tag="ang")
                for tau in range(GRP):
                    t = t0 + tau
                    if tau < na:
                        # path A: fused scale inside the Sin activation
                        nc.scalar.activation(
                            sino[:, tau * 256 : (tau + 1) * 256],
                            invp2g[:],
                            AF.Sin,
                            bias=bias_hp[:],
                            scale=k2pi[:, t : t + 1],
                        )
                    else:
                        # path B: angle on VectorE, sin batched below
                        b = tau - na
                        nc.vector.tensor_scalar(
                            ang[:, b * 256 : (b + 1) * 256],
                            invp2g[:],
                            k2pi[:, t : t + 1],
                            None,
                            OP.mult,
                        )
                if nb > 0:
                    nc.scalar.activation(
                        sino[:, na * 256 :],
                        ang[:],
                        AF.Sin,
                        bias=bias_hp[:],
                        scale=1.0,
                    )
                # prodx layout: (glo, tt, i) so each glo slice is contiguous
                prodx = prodp.tile([128, GRP * 256], f16, tag="prodx")
                pm4 = prodx[:].rearrange(
                    "p (glo tt i) -> p tt glo i", glo=4, tt=GRP
                )
                for tau in range(GRP):
                    t = t0 + tau
                    if USE_STT:
                        nc.vector.scalar_tensor_tensor(
                            pm4[:, tau, :, :],
                            sino[:, tau * 256 : (tau + 1) * 256].rearrange(
                                "p (glo i) -> p glo i", glo=4
                            ),
                            xprep[:, t : t + 1],
                            p2hg[:].rearrange("p (glo i) -> p glo i", glo=4),
                            OP.mult,
                            OP.mult,
                        )
                    else:
                        sx = sinp.tile([128, 256], f16, tag="sx")
                        nc.vector.tensor_scalar(
                            sx[:],
                            sino[:, tau * 256 : (tau + 1) * 256],
                            xprep[:, t : t + 1],
                            None,
                            OP.mult,
                        )
                        nc.vector.tensor_tensor(
                            pm4[:, tau, :, :],
                            sx[:].rearrange("p (glo i) -> p glo i", glo=4),
                            p2hg[:].rearrange("p (glo i) -> p glo i", glo=4),
                            OP.mult,
                        )
                # PE: partition-sum over p, accumulating the 4 glo slices
                # into this group's PSUM row
                rows = prowp.tile([1, GRP * D], f32, tag="rows")
                for glo in range(4):
                    nc.tensor.matmul(
                        rows[:],
                        ones_h[:],
                        prodx[:, glo * GRP * D : (glo + 1) * GRP * D],
                        start=(glo == 0),
                        stop=(glo == 3),
                    )
                seg = rows_sb[0:1, g * GRP * D : (g + 1) * GRP * D]
                if g % 2 == 0:
                    nc.scalar.copy(seg, rows[:])
                else:
                    nc.vector.tensor_copy(seg, rows[:])

            # ---------------- combine + output ----------------
            # row buffer is y-flat; SBUF->SBUF partition-scatter DMA is
            # broken in this stack, so bounce through an HBM scratch in
            # 4 chunks (each can start as soon as its quarter is done)
            CH = TOK * D // 4
            for c in range(4):
                nc.sync.dma_start(
                    out=scratch[c * CH : (c + 1) * CH].rearrange(
                        "(a f) -> a f", a=1
                    ),
                    in_=rows_sb[0:1, c * CH : (c + 1) * CH],
                )
                nc.sync.dma_start(
                    out=out_sb[c * (TOK // 4) : (c + 1) * (TOK // 4), :],
                    in_=scratch[c * CH : (c + 1) * CH].rearrange(
                        "(t i) -> t i", i=D
                    ),
                )
            nc.vector.tensor_tensor(out_sb[:], out_sb[:], resT[:], OP.add)
            nc.vector.tensor_tensor(
                out_sb[:, 0:4], out_sb[:, 0:4], nksm[:], OP.add
            )
            nc.sync.dma_start(out=Y[:], in_=out_sb[:])

    if split:
        _split_waits(nc)
    return nc


def kernel(x, k, M, R, P, gamma, beta, periods):
    from concourse.bass_utils import run_bass_kernel_spmd

    if "nc" not in _CACHE:
        _CACHE["nc"] = _build_program()
    nc = _CACHE["nc"]

    xf = np.ascontiguousarray(x, dtype=np.float32).reshape(B * S, D)
    kf = np.ascontiguousarray(k, dtype=np.float32).reshape(B * S)
    Mf = np.ascontiguousarray(M, dtype=np.float32)
    Rf = np.ascontiguousarray(R, dtype=np.float32)
    Pf = np.ascontiguousarray(P, dtype=np.float32).reshape(-1)
    gf = np.ascontiguousarray(gamma, dtype=np.float32).reshape(1, D)
    bf = np.ascontiguousarray(beta, dtype=np.float32).reshape(1, D)
    pf = np.ascontiguousarray(periods, dtype=np.float32).reshape(-1)

    ones = np.ones((1, 128), dtype=np.float32)
    repm = np.zeros((D, 128), dtype=np.float32)
    repm[np.arange(128) // 2, np.arange(128)] = 1.0
    idm = np.eye(128, dtype=np.float32)

    in_maps = []
    for core in range(N_CORES):
        sl = slice(core * TOK, (core + 1) * TOK)
        in_maps.append(
            {
                "x": xf[sl],
                "k": kf[sl].reshape(1, TOK),
                "M": Mf,
                "R": Rf,
                "P": Pf,
                "gamma": gf,
                "beta": bf,
                "periods": pf,
                "ones": ones,
                "rep": repm,
                "idm": idm,
            }
        )

    _CACHE["in_maps"] = in_maps
    res = run_bass_kernel_spmd(nc, in_maps, core_ids=list(range(N_CORES)))
    out = np.concatenate([res.results[c]["y"] for c in range(N_CORES)], axis=0)
    return out.reshape(B, S, D)



# revision 10
# speedup vs baseline: 2.4234x; 2.4234x over previous
import math

import numpy as np

# nn_DescLayer, period-sharded design.
#
# y[t,i] = res[t,i] + sum_{j,g} P[i,j,g] cos(2pi k_t / per[i,j,g]) xp[t,j]
#        = res[t,i] + sum_j W[k_t, i, j] xp[t,j],
#   W[k,i,j] = sum_g P[i,j,g] cos(k * w_g),  w_g = 2pi/per[i,j,g].
#
# Sharding: core c owns output columns i in {c, c+8, ..., c+56}; every core
# processes all 1024 tokens (no cross-core communication).
#
# Key reduction (8x fewer trig evals): for per >= ~240, linearize over g
# around what = mean_g(w_g):
#   W[k,i,j] ~= cos(k*what)*A[i,j] - k*sin(k*what)*B[i,j],
#   A = sum_g P, B = sum_g P*(w_g - what).
# Exact per-g evaluation (with mod-1 range reduction) only for the first
# EXJ j's of i_loc=0 (smallest periods). Validated: end-to-end rel err
# ~4e-4 vs fp64 reference.
#
# Tables are computed with k on partitions (k = kc*128 + p, kc=0..3), ij on
# the free dim, so the ScalarE Sin activation with per-partition scale=k
# batches 128 k-values per instruction. The per-token "gather" W[k_t] is a
# PE one-hot matmul: stationary U[k, t] (host-built one-hot), moving
# W2[k, ij] fp16, accumulated over the 4 k-chunks into PSUM [t, ij] per
# 128-token group. Combine = broadcast-mult by xp (stride-0 AP views) +
# free-dim reduce over j.

B, S, D, NB = 2, 512, 64, 8
N_CORES = 8
NT = B * S  # 1024 tokens, all on every core
NK = 512  # k values
NKC = 4  # k chunks of 128
IJ = 512  # (i_loc, j) entries per core
SMJ = 64  # i_loc=0 columns (linearized via mod-1 reduction)
EXJ = 30  # exact j's (i_loc=0, j<EXJ)
EXC = EXJ * NB  # 240 exact flat columns
REGC = IJ - SMJ  # 448 regular columns (direct LUT)
NTG = NT // 128  # 8 token groups
LN_EPS = 1e-5
RND_C = 12582912.0  # 1.5*2^23: (u+C)-C == round-to-nearest(u) in f32
TWO_PI = 2.0 * math.pi

_CACHE = {}


def _split_waits(nc, maxw=1):
    """This walrus build rejects instructions carrying more than one sem
    wait. Hoist excess waits onto same-engine NoOps placed immediately
    before the instruction (same engine stream => executes first)."""
    import bass_rust
    import concourse.mybir as mybir

    ctr = [0]
    for f in nc.m.functions:
        for b in f.blocks:
            new_insts = []
            changed = False
            for inst in b.instructions:
                si = inst.sync_info
                waits = list(si.on_wait) if si and si.on_wait else []
                if len(waits) > maxw:
                    keep = waits[-maxw:]
                    extra = waits[:-maxw]
                    for i0 in range(0, len(extra), maxw):
                        ctr[0] += 1
                        nop = bass_rust.InstNoOp(
                            name=f"I-waitsplit-{ctr[0]}",
                            engine=inst.engine,
                            text_hint="waitsplit",
                            sync_info=mybir.SyncInfo(
                                on_wait=extra[i0 : i0 + maxw], on_update=[]
                            ),
                        )
                        new_insts.append(nop)
                    si.on_wait = keep
                    changed = True
                new_insts.append(inst)
            if changed:
                b.instructions = new_insts


def _build_program():
    import concourse.bass as bass
    import concourse.mybir as mybir
    from concourse.tile import TileContext
    from concourse.vector_clock import ScopedClock, VectorClock

    # walrus also rejects the multi-wait Tile tail drain; spread the waits
    # over SP nops (1 each), then issue a bare drain.
    def _drain_and_barrier(self, tick_clock, wait_clock):
        nc = self.nc
        gc = tick_clock.global_clock
        n = len(gc)
        for i in range(n):
            tick = gc[i]
            if tick <= 0:
                continue
            vec = [0] * n
            vec[i] = tick
            nop_inst = nc.sync.nop(nofuse=True, hint=f"drain_wait_{i}")
            wait_clock.add_sem_waits(
                nop_inst.ins, ScopedClock({None: VectorClock(vec)})
            )
        nc.sync.drain()
        nc.all_engine_barrier()
        assert self.sems is not None
        popped = nc._tile_sem_poison_stack.pop()
        assert popped is self._sem_poison
        nc.clear_and_free_semaphores(list(self.sems.allocated().values()))
        nc.all_engine_barrier()

    TileContext._drain_and_barrier = _drain_and_barrier

    f32 = mybir.dt.float32
    f16 = mybir.dt.float16
    AF = mybir.ActivationFunctionType
    OP = mybir.AluOpType
    AX = mybir.AxisListType

    nc = bass.Bass()
    X = nc.declare_dram_parameter("x", [128, NTG * D], f32, isOutput=False)
    U = nc.declare_dram_parameter("u1h", [128, NKC * NT], f16, isOutput=False)
    KC = nc.declare_dram_parameter("kcols", [128, 2 * NKC], f32, isOutput=False)
    WH = nc.declare_dram_parameter("whatc", [1, REGC], f32, isOutput=False)
    VIN = nc.declare_dram_parameter("vin", [1, EXC + SMJ], f32, isOutput=False)
    AC = nc.declare_dram_parameter("acol", [1, IJ], f32, isOutput=False)
    BC = nc.declare_dram_parameter("bcol", [1, IJ], f32, isOutput=False)
    P0 = nc.declare_dram_parameter("p0", [1, EXC], f32, isOutput=False)
    MG = nc.declare_dram_parameter("mgt", [D, D], f32, isOutput=False)
    RG = nc.declare_dram_parameter("rgt", [D, 8], f32, isOutput=False)
    MB = nc.declare_dram_parameter("mbeta", [1, D], f32, isOutput=False)
    RB = nc.declare_dram_parameter("rbeta", [1, 8], f32, isOutput=False)
    IDM = nc.declare_dram_parameter("idm", [128, 128], f32, isOutput=False)
    Y = nc.declare_dram_parameter("y", [128, NTG * 8], f32, isOutput=True)

    with TileContext(nc) as tc:
        with (
            tc.tile_pool(name="const", bufs=1) as cp,
            tc.tile_pool(name="tab", bufs=2) as tp,
            tc.tile_pool(name="wrow", bufs=1, space="PSUM") as wrp,
            tc.tile_pool(name="pprep", bufs=1, space="PSUM") as pp,
        ):
            # ---------------- input DMAs ----------------
            xs = cp.tile([128, NTG, D], f32, tag="xs")
            nc.sync.dma_start(out=xs[:], in_=X[:].rearrange("p (t j) -> p t j", j=D))
            u1h = cp.tile([128, NKC * NT], f16, tag="u1h")
            nc.sync.dma_start(out=u1h[:], in_=U[:])
            kcols = cp.tile([128, 2 * NKC], f32, tag="kcols")
            nc.sync.dma_start(out=kcols[:], in_=KC[:])
            whatc = cp.tile([128, REGC], f32, tag="whatc")
            nc.sync.dma_start(out=whatc[:], in_=WH[:].to_broadcast((128, REGC)))
            vin = cp.tile([128, EXC + SMJ], f32, tag="vin")
            nc.sync.dma_start(out=vin[:], in_=VIN[:].to_broadcast((128, EXC + SMJ)))
            acol = cp.tile([128, IJ], f32, tag="acol")
            nc.sync.dma_start(out=acol[:], in_=AC[:].to_broadcast((128, IJ)))
            bcol = cp.tile([128, IJ], f32, tag="bcol")
            nc.sync.dma_start(out=bcol[:], in_=BC[:].to_broadcast((128, IJ)))
            p0b = cp.tile([128, EXC], f32, tag="p0b")
            nc.sync.dma_start(out=p0b[:], in_=P0[:].to_broadcast((128, EXC)))
            mgt = cp.tile([D, D], f32, tag="mgt")
            nc.sync.dma_start(out=mgt[:], in_=MG[:])
            rgt = cp.tile([D, 8], f32, tag="rgt")
            nc.sync.dma_start(out=rgt[:], in_=RG[:])
            mbet = cp.tile([128, D], f32, tag="mbet")
            nc.sync.dma_start(out=mbet[:], in_=MB[:].to_broadcast((128, D)))
            rbet = cp.tile([128, 8], f32, tag="rbet")
            nc.sync.dma_start(out=rbet[:], in_=RB[:].to_broadcast((128, 8)))
            idm = cp.tile([128, 128], f32, tag="idm")
            nc.sync.dma_start(out=idm[:], in_=IDM[:])

            # ---------------- LayerNorm (token-major, pure normalize) -----
            # gamma is folded into mgt/rgt on host; beta enters via mbeta/
            # rbeta adds below.
            xsq = cp.tile([128, NTG, D], f32, tag="xsq")
            nc.scalar.activation(xsq[:], xs[:], AF.Square, scale=1.0)
            rsum = cp.tile([128, NTG], f32, tag="rsum")
            nc.vector.tensor_reduce(rsum[:], xs[:], AX.X, OP.add)
            ssq = cp.tile([128, NTG], f32, tag="ssq")
            nc.vector.tensor_reduce(ssq[:], xsq[:], AX.X, OP.add)
            mu = cp.tile([128, NTG], f32, tag="mu")
            nc.vector.tensor_scalar(mu[:], rsum[:], 1.0 / D, None, OP.mult)
            mu2 = cp.tile([128, NTG], f32, tag="mu2")
            nc.vector.tensor_tensor(mu2[:], mu[:], mu[:], OP.mult)
            veps = cp.tile([128, NTG], f32, tag="veps")
            nc.vector.scalar_tensor_tensor(
                veps[:], ssq[:], 1.0 / D, mu2[:], OP.mult, OP.subtract
            )
            vep2 = cp.tile([128, NTG], f32, tag="vep2")
            nc.vector.tensor_scalar(vep2[:], veps[:], LN_EPS, None, OP.add)

            # rstd = 1/sqrt(vep2): bit-hack seed + 2 Newton steps (keeps the
            # ScalarE activation table on the trig set)
            i32 = mybir.dt.int32
            ti = cp.tile([128, NTG], i32, tag="ti")
            nc.vector.tensor_scalar(
                ti[:], vep2[:].bitcast(i32), 1, -1, OP.arith_shift_right,
                OP.bitwise_xor,
            )
            yr = cp.tile([128, NTG], f32, tag="yr")
            nc.vector.tensor_scalar(
                yr[:].bitcast(i32), ti[:], 0x5F3759DF + 1, None, OP.add
            )
            hh = cp.tile([128, NTG], f32, tag="hh")
            nc.vector.tensor_scalar(hh[:], vep2[:], 0.5, None, OP.mult)
            for it in range(2):
                t1 = cp.tile([128, NTG], f32, tag=f"nw1_{it}")
                nc.vector.tensor_tensor(t1[:], yr[:], yr[:], OP.mult)
                t2 = cp.tile([128, NTG], f32, tag=f"nw2_{it}")
                nc.vector.tensor_tensor(t2[:], t1[:], hh[:], OP.mult)
                t3 = cp.tile([128, NTG], f32, tag=f"nw3_{it}")
                nc.vector.tensor_scalar(t3[:], t2[:], 1.5, -1.0, OP.subtract, OP.mult)
                yn = cp.tile([128, NTG], f32, tag=f"nw4_{it}")
                nc.vector.tensor_tensor(yn[:], yr[:], t3[:], OP.mult)
                yr = yn

            cen = cp.tile([128, NTG, D], f32, tag="cen")
            nc.vector.tensor_tensor(
                cen[:], xs[:], mu[:].rearrange("p (t a) -> p t a", a=1).broadcast_to([128, NTG, D]),
                OP.subtract,
            )
            lnf = cp.tile([128, NTG, D], f32, tag="lnf")
            nc.vector.tensor_tensor(
                lnf[:], cen[:], yr[:].rearrange("p (t a) -> p t a", a=1).broadcast_to([128, NTG, D]),
                OP.mult,
            )

            # ---------------- lnT + projections ----------------
            # lnT[j, t] per token-group via PE transpose, then per-group
            # stationary lnT-slice feeds xp_tok and res_tok matmuls.
            lnT_ps = pp.tile([D, NT], f32, tag="lnT_ps")
            for tg in range(NTG):
                nc.tensor.transpose(
                    lnT_ps[:, tg * 128 : (tg + 1) * 128], lnf[:, tg, :], idm[:]
                )
            lnT = cp.tile([D, NT], f16, tag="lnT")
            nc.scalar.copy(lnT[:], lnT_ps[:])

            mgt16 = cp.tile([D, D], f16, tag="mgt16")
            nc.vector.tensor_copy(mgt16[:], mgt[:])
            rgt16 = cp.tile([D, 8], f16, tag="rgt16")
            nc.vector.tensor_copy(rgt16[:], rgt[:])

            xp_ps = pp.tile([128, NTG, D], f32, tag="xp_ps")
            res_ps = pp.tile([128, NTG, 8], f32, tag="res_ps")
            for tg in range(NTG):
                sl = lnT[:, tg * 128 : (tg + 1) * 128]
                nc.tensor.matmul(
                    xp_ps[:, tg, :], sl, mgt16[:], start=True, stop=True
                )
                nc.tensor.matmul(
                    res_ps[:, tg, :], sl, rgt16[:], start=True, stop=True
                )
            xp_tok = cp.tile([128, NTG, D], f16, tag="xp_tok")
            nc.vector.tensor_tensor(
                xp_tok[:], xp_ps[:],
                mbet[:].rearrange("p (a j) -> p a j", a=1).broadcast_to([128, NTG, D]),
                OP.add,
            )

            # ---------------- tables: sinT/cosT [128k, (kc, ij)] ----------
            # regular cols (ij >= SMJ): direct LUT, angle = k*what <= 1.566
            sinT = tp.tile([128, NKC, IJ], f16, tag="sinT")
            cosT = tp.tile([128, NKC, IJ], f16, tag="cosT")
            bias_hp = cp.tile([128, 1], f32, tag="bias_hp")
            nc.vector.memset(bias_hp[:], math.pi / 2.0)
            bias_z = cp.tile([128, 1], f32, tag="bias_z")
            nc.vector.memset(bias_z[:], 0.0)
            for kc in range(NKC):
                k2p = kcols[:, kc : kc + 1]  # k (plain)
                nc.scalar.activation(
                    sinT[:, kc, SMJ:], whatc[:], AF.Sin, bias=bias_z[:], scale=k2p
                )
                nc.scalar.activation(
                    cosT[:, kc, SMJ:], whatc[:], AF.Sin, bias=bias_hp[:], scale=k2p
                )

            # V path: frac(k*inv + 0.25) for [exact EXC cols | smalli SMJ
            # cols]; sin(2pi f) = cos(2pi k inv).
            NV = EXC + SMJ
            u4v = tp.tile([128, NKC, NV], f32, tag="u4v")
            for kc in range(NKC):
                nc.vector.tensor_scalar(
                    u4v[:, kc, :], vin[:], kcols[:, kc : kc + 1], 0.25, OP.mult, OP.add
                )
            r4v = tp.tile([128, NKC, NV], f32, tag="r4v")
            nc.vector.tensor_scalar(r4v[:], u4v[:], RND_C, RND_C, OP.add, OP.subtract)
            f4v = tp.tile([128, NKC, NV], f32, tag="f4v")
            nc.vector.tensor_tensor(f4v[:], u4v[:], r4v[:], OP.subtract)
            # exact cols -> vcos (feeds P0 mult+reduce); smalli cols -> cosT
            vcos = tp.tile([128, NKC, EXC], f16, tag="vcos")
            nc.scalar.activation(
                vcos[:], f4v[:, :, :EXC], AF.Sin, bias=bias_z[:], scale=TWO_PI
            )
            nc.scalar.activation(
                cosT[:, :, :SMJ], f4v[:, :, EXC:], AF.Sin, bias=bias_z[:], scale=TWO_PI
            )

            # U path: frac(k*inv) for smalli cols -> sinT
            u4u = tp.tile([128, NKC, SMJ], f32, tag="u4u")
            for kc in range(NKC):
                nc.vector.tensor_scalar(
                    u4u[:, kc, :], vin[:, EXC:], kcols[:, kc : kc + 1], None, OP.mult
                )
            r4u = tp.tile([128, NKC, SMJ], f32, tag="r4u")
            nc.vector.tensor_scalar(r4u[:], u4u[:], RND_C, RND_C, OP.add, OP.subtract)
            f4u = tp.tile([128, NKC, SMJ], f32, tag="f4u")
            nc.vector.tensor_tensor(f4u[:], u4u[:], r4u[:], OP.subtract)
            nc.scalar.activation(
                sinT[:, :, :SMJ], f4u[:], AF.Sin, bias=bias_z[:], scale=TWO_PI
            )

            # ---------------- W2 [128k, (kc, ij)] fp16 ----------------
            # W2 = cosT*A - (k/512)*sinT*(512*B); exact override on ij<EXJ*?
            w2 = tp.tile([128, NKC, IJ], f16, tag="w2")
            i1 = tp.tile([128, NKC, IJ], f16, tag="i1")
            for kc in range(NKC):
                nc.vector.scalar_tensor_tensor(
                    i1[:, kc, :], sinT[:, kc, :], kcols[:, NKC + kc : NKC + kc + 1],
                    bcol[:], OP.mult, OP.mult,
                )
            nc.vector.tensor_tensor(
                w2[:], cosT[:],
                acol[:].rearrange("p (a c) -> p a c", a=1).broadcast_to([128, NKC, IJ]),
                OP.mult,
            )
            nc.vector.tensor_tensor(w2[:], w2[:], i1[:], OP.subtract)
            # exact override: W2[:, kc, j<EXJ] = sum_g P0 * vcos
            pc0 = tp.tile([128, NKC, EXC], f16, tag="pc0")
            nc.vector.tensor_tensor(
                pc0[:], vcos[:],
                p0b[:].rearrange("p (a c) -> p a c", a=1).broadcast_to([128, NKC, EXC]),
                OP.mult,
            )
            with nc.allow_low_precision(reason="8-way fp16 g-sum, validated 4.8e-4 rel err"):
                nc.vector.tensor_reduce(
                    w2[:, :, :EXJ],
                    pc0[:].rearrange("p kc (j g) -> p kc j g", g=NB),
                    AX.X, OP.add,
                )

            # ---------------- one-hot gather on PE + combine --------------
            # Wrow[t, ij] = sum_k U[k, t] W2[k, ij], two PSUM waves of 4
            # token-groups; combine = mult by xp (stride-0 bcast) + reduce_j.
            nk_tok = cp.tile([128, NTG, 8], f32, tag="nk_tok")
            for wv in range(2):
                wrow = wrp.tile([128, 4, IJ], f32, tag="wrow")
                for tgi in range(4):
                    tg = wv * 4 + tgi
                    for kc in range(NKC):
                        nc.tensor.matmul(
                            wrow[:, tgi, :],
                            u1h[:, kc * NT + tg * 128 : kc * NT + (tg + 1) * 128],
                            w2[:, kc, :],
                            start=(kc == 0),
                            stop=(kc == NKC - 1),
                        )
                m = tp.tile([128, 4, 8, D], f16, tag="m")
                nc.vector.tensor_tensor(
                    m[:],
                    wrow[:].rearrange("p t (i j) -> p t i j", j=D),
                    xp_tok[:, wv * 4 : wv * 4 + 4, :]
                    .rearrange("p t (a j) -> p t a j", a=1)
                    .broadcast_to([128, 4, 8, D]),
                    OP.mult,
                )
                nc.vector.tensor_reduce(
                    nk_tok[:, wv * 4 : wv * 4 + 4, :], m[:], AX.X, OP.add
                )

            # ---------------- output ----------------
            y0 = cp.tile([128, NTG, 8], f32, tag="y0")
            nc.vector.tensor_tensor(y0[:], nk_tok[:], res_ps[:], OP.add)
            yout = cp.tile([128, NTG, 8], f32, tag="yout")
            nc.vector.tensor_tensor(
                yout[:], y0[:],
                rbet[:].rearrange("p (a i) -> p a i", a=1).broadcast_to([128, NTG, 8]),
                OP.add,
            )
            nc.sync.dma_start(
                out=Y[:], in_=yout[:].rearrange("p t i -> p (t i)")
            )

    _split_waits(nc)
    return nc


def kernel(x, k, M, R, P, gamma, beta, periods):
    from concourse.bass_utils import run_bass_kernel_spmd

    if "nc" not in _CACHE:
        _CACHE["nc"] = _build_program()
    nc = _CACHE["nc"]

    xf = np.ascontiguousarray(x, dtype=np.float32).reshape(NT, D)
    kf = np.ascontiguousarray(k, dtype=np.int64).reshape(NT)
    Mf = np.asarray(M, dtype=np.float64)
    Rf = np.asarray(R, dtype=np.float64)
    Pf = np.asarray(P, dtype=np.float64)
    gf = np.asarray(gamma, dtype=np.float64)
    bf = np.asarray(beta, dtype=np.float64)
    perf = np.asarray(periods, dtype=np.float64)

    # token-major x: token t -> (partition t%128, group t//128)
    x_sb = xf.reshape(NTG, 128, D).transpose(1, 0, 2).reshape(128, NTG * D)
    x_sb = np.ascontiguousarray(x_sb)

    # one-hot U[k, t] blocked by k-chunk: u1h[p, kc*NT + t] = (k_t == kc*128+p)
    u1h = np.zeros((128, NKC * NT), dtype=np.float16)
    t_idx = np.arange(NT)
    u1h[kf % 128, (kf // 128) * NT + t_idx] = 1.0

    # k scalar columns: [0:4] = k, [4:8] = k/512
    pcol = np.arange(128, dtype=np.float64)
    kcols = np.zeros((128, 2 * NKC), dtype=np.float32)
    for kc in range(NKC):
        kcols[:, kc] = pcol + 128 * kc
        kcols[:, NKC + kc] = (pcol + 128 * kc) / 512.0

    idm = np.eye(128, dtype=np.float32)

    w = TWO_PI / perf  # (i, j, g)
    what = w.mean(-1)  # (i, j)
    A = Pf.sum(-1)  # (i, j)
    Bc = 512.0 * (Pf * (w - what[..., None])).sum(-1)  # (i, j), pre-scaled

    in_maps = []
    for c in range(N_CORES):
        i_set = np.arange(8) * 8 + c  # i_loc -> global i
        what_c = what[i_set]  # (8, 64)
        A_c = A[i_set]
        B_c = Bc[i_set]
        # regular cols: ij >= SMJ, i.e. i_loc >= 1
        whatc = what_c[1:].reshape(1, REGC).astype(np.float32)
        acol = A_c.reshape(1, IJ).astype(np.float32)
        bcol = B_c.reshape(1, IJ).astype(np.float32)
        # V/U path inputs: [exact 1/per (EXC) | smalli what/(2pi) (SMJ)]
        per0 = perf[i_set[0], :EXJ, :].reshape(EXC)  # exact periods
        invs = what_c[0] / TWO_PI  # (64,) cycles-per-k for i_loc = 0
        vin = np.concatenate([1.0 / per0, invs]).reshape(1, EXC + SMJ).astype(np.float32)
        p0 = Pf[i_set[0], :EXJ, :].reshape(1, EXC).astype(np.float32)
        # projections: fold gamma into M/R, beta via M@beta / R@beta
        mgt = (Mf * gf[None, :]).T.astype(np.float32)  # (j, o)
        mbeta = (Mf @ bf).reshape(1, D).astype(np.float32)
        rgt = (Rf[i_set] * gf[None, :]).T.astype(np.float32)  # (j, 8)
        rbeta = (Rf[i_set] @ bf).reshape(1, 8).astype(np.float32)
        in_maps.append(
            {
                "x": x_sb,
                "u1h": u1h,
                "kcols": kcols,
                "whatc": np.ascontiguousarray(whatc),
                "vin": np.ascontiguousarray(vin),
                "acol": np.ascontiguousarray(acol),
                "bcol": np.ascontiguousarray(bcol),
                "p0": np.ascontiguousarray(p0),
                "mgt": np.ascontiguousarray(mgt),
                "rgt": np.ascontiguousarray(rgt),
                "mbeta": mbeta,
                "rbeta": rbeta,
                "idm": idm,
            }
        )

    _CACHE["in_maps"] = in_maps
    res = run_bass_kernel_spmd(nc, in_maps, core_ids=list(range(N_CORES)))
    # y[p, (t i)] per core -> y_full[t, i_set]
    out = np.empty((NT, D), dtype=np.float32)
    for c in range(N_CORES):
        yc = res.results[c]["y"].reshape(128, NTG, 8)  # (p, tg, i_loc)
        i_set = np.arange(8) * 8 + c
        out[:, i_set] = yc.transpose(1, 0, 2).reshape(NT, 8)
    return out.reshape(B, S, D)
